# revision 22
# baseline (speedup 1.0000x reference)
"""Trainium2 Bass kernel for BitLTIInjection (BitNet-style fake-quantized linear
+ LTI injection):

    A_eff = 0.99*tanh(A_raw)
    e_q   = per-token absmax int8 fake quant of e
    W_q   = absmean ternary fake quant of W
    out   = A_eff*h + e_q @ W_q.T + block_out

Data-parallel over B*T across 8 cores; W replicated.  The quantized matmul
runs in bf16 (quantized values are small integers, so bf16 matmul with fp32
PSUM accumulation is numerically exact); dequant scales fold into the
PSUM->SBUF epilogue.  Rounding uses the f32 magic-number trick.

v2 schedule: W streams in first at full HBM rate while DVE computes the
absmean; ternarize+transpose runs per 4-row-tile output group, pipelined
against the matmul stream (group g's weights prep during group g-1's
matmuls).  Matmul blocks run in an order that gives the e-quant pipeline
runway: (g0,i0-7),(g1,i0-7),(g0,i8-15),(g1,i8-15),(g2,*),(g3,*).
Engine split: SP=loads, ACT=f32->bf16 passes + transposes, DVE=scale math +
epilogues, Pool=e reduces + W reloads + out stores, PE=matmuls.
"""

import numpy as np

import concourse.bass as bass
import concourse.mybir as mybir
import concourse.tile as tile
from concourse.bass import ts
from concourse.tile_rust import add_dep_helper
from concourse.bass_utils import run_bass_kernel_spmd

P = 128
MAGIC = 12582912.0  # 1.5 * 2**23: forces RNE-to-integer in f32
EPS = 1e-5
N_CORES = 8
F32 = mybir.dt.float32
BF16 = mybir.dt.bfloat16
MM_N = 512  # moving free dim per matmul (one PSUM bank of f32)


def build_kernel_body(tc: tile.TileContext, io: dict, Tc: int, D: int, with_h: bool):
    nc = tc.nc
    n_tb = Tc // P      # token blocks per core (16)
    n_dc = D // P       # contraction chunks (16)
    n_ob = D // MM_N    # output column groups (4)
    n_wt = D // P       # weight row tiles (16)
    GJ = n_wt // n_ob   # W row tiles per output group (4)

    e_d = io["e"]
    bo_d = io["bo"]
    w_d = io["w"]
    out_d = io["out"]

    resident_g0 = not with_h  # keep last GJ W f32 tiles resident -> no g0 reload
    wf_bufs = (GJ + 0) if resident_g0 else 2

    with (
        tc.tile_pool(name="wqt", bufs=1) as wqt_pool,
        tc.tile_pool(name="et", bufs=1) as et_pool,
        tc.tile_pool(name="wf", bufs=wf_bufs) as wf_pool,
        tc.tile_pool(name="wb", bufs=2) as wb_pool,
        tc.tile_pool(name="ef", bufs=2) as ef_pool,
        tc.tile_pool(name="qb", bufs=1) as qb_pool,
        tc.tile_pool(name="bo", bufs=5) as bo_pool,
        tc.tile_pool(name="scal", bufs=1) as scal_pool,
        tc.tile_pool(name="pp", bufs=6, space="PSUM") as pp_pool,
        tc.tile_pool(name="tp", bufs=2, space="PSUM") as tp_pool,
    ):
        # resident transposed ternary weights [d0, dc, o] and transposed e
        wqt = wqt_pool.tile([P, n_dc, D], BF16, tag="wqt")
        eT = et_pool.tile([P, n_tb, n_dc, P], BF16, tag="eT")

        ones_col = scal_pool.tile([P, 1], F32, tag="ones_col")
        nc.vector.memset(ones_col[:], 1.0)
        ones_row = scal_pool.tile([1, P], F32, tag="ones_row")
        nc.vector.memset(ones_row[:], 1.0)
        negmagic = scal_pool.tile([P, 1], F32, tag="negmagic")
        nc.vector.memset(negmagic[:], -MAGIC)
        # identity (bf16) for PE-transposes of the ternarized weights
        ident = scal_pool.tile([P, P], BF16, tag="ident")
        nc.vector.memset(ident[:], 1.0)
        nc.gpsimd.affine_select(
            ident[:], ident[:], pattern=[[1, P]],
            compare_op=mybir.AluOpType.is_equal, fill=0.0,
            base=0, channel_multiplier=-1,
        )

        # ---------------- W load (full-rate) + absmean ----------------
        # per-tile |W| row sums on DVE (idle during the load phase)
        parts = scal_pool.tile([P, n_wt], F32, tag="parts")
        if resident_g0:
            load_order = list(range(GJ, n_wt)) + list(range(GJ))
        else:
            load_order = list(range(n_wt))
        wf_tiles = {}
        w_load_ins = []
        for j in load_order:
            wf = wf_pool.tile([P, D], F32, tag="wf", name=f"wfm_{j}")
            w_load_ins.append(nc.sync.dma_start(out=wf[:], in_=w_d[ts(j, P), :]))
            nc.vector.tensor_reduce(
                out=parts[:, j : j + 1],
                in_=wf[:],
                axis=mybir.AxisListType.X,
                op=mybir.AluOpType.add,
                apply_absolute_value=True,
            )
            wf_tiles[j] = wf

        hp = tc.high_priority()
        hp.__enter__()
        acc = scal_pool.tile([P, 1], F32, tag="acc")
        nc.vector.tensor_reduce(
            out=acc[:], in_=parts[:], axis=mybir.AxisListType.X,
            op=mybir.AluOpType.add,
        )
        # cross-partition sum + broadcast via tiny PE matmuls
        tot_ps = pp_pool.tile([P, MM_N], F32, tag="ps", name="tot_ps")
        nc.tensor.matmul(tot_ps[:1, :1], ones_col[:], acc[:])
        tot_sb = scal_pool.tile([1, 1], F32, tag="tot_sb")
        nc.vector.tensor_copy(out=tot_sb[:], in_=tot_ps[:1, :1])
        asum_ps = pp_pool.tile([P, MM_N], F32, tag="ps", name="asum_ps")
        nc.tensor.matmul(asum_ps[:, :1], ones_row[:], tot_sb[:])
        allsum = scal_pool.tile([P, 1], F32, tag="allsum")
        nc.vector.tensor_copy(out=allsum[:], in_=asum_ps[:, :1])
        # m = max(mean_abs, EPS); s_w = 1/m (Newton-refined); deqm = m/127
        m_t = scal_pool.tile([P, 1], F32, tag="m_t")
        nc.vector.tensor_scalar(
            out=m_t[:], in0=allsum[:], scalar1=1.0 / (D * D), scalar2=EPS,
            op0=mybir.AluOpType.mult, op1=mybir.AluOpType.max,
        )
        r0w = scal_pool.tile([P, 1], F32, tag="r0w")
        nc.vector.reciprocal(r0w[:], m_t[:])
        t1w = scal_pool.tile([P, 1], F32, tag="t1w")
        nc.vector.scalar_tensor_tensor(
            out=t1w[:], in0=m_t[:], scalar=-1.0, in1=r0w[:],
            op0=mybir.AluOpType.mult, op1=mybir.AluOpType.mult,
        )
        nc.vector.tensor_scalar_add(t1w[:], t1w[:], 2.0)
        s_w = scal_pool.tile([P, 1], F32, tag="s_w")
        nc.vector.tensor_scalar_mul(s_w[:], r0w[:], t1w[:])
        deqm = scal_pool.tile([P, 1], F32, tag="deqm")
        nc.vector.tensor_scalar_mul(deqm[:], m_t[:], 1.0 / 127.0)
        hp.__exit__(None, None, None)

        # ---------------- A_eff (only if nonzero A_raw) ----------------
        if with_h:
            a_d = io["a_raw"]
            a1 = scal_pool.tile([1, D], F32, tag="a1")
            nc.sync.dma_start(out=a1[:], in_=a_d[:, :])
            aeff = scal_pool.tile([P, D], F32, tag="aeff")
            for ob in range(n_ob):
                ab_ps = pp_pool.tile([P, MM_N], F32, tag="ps", name=f"ab_ps{ob}")
                nc.tensor.matmul(ab_ps[:], ones_row[:], a1[:, ts(ob, MM_N)])
                nc.vector.tensor_copy(out=aeff[:, ts(ob, MM_N)], in_=ab_ps[:])
            nc.scalar.activation(
                aeff[:], aeff[:], mybir.ActivationFunctionType.Tanh
            )
            nc.vector.tensor_scalar_mul(aeff[:], aeff[:], 0.99)

        # ---------------- W ternarize helpers ----------------
        def w_reload(j):
            wf = wf_pool.tile([P, D], F32, tag="wf", name=f"wfr_{j}")
            nc.sync.dma_start(out=wf[:], in_=w_d[ts(j, P), :])
            wf_tiles[j] = wf

        def w_prep(j, passb_engine):
            """round(W*s_w) -> bf16, transpose into wqt, clip later."""
            wf = wf_tiles[j]
            # passA in place: wf = wf*s_w + MAGIC  (rounds to int, biased)
            nc.vector.tensor_scalar(
                out=wf[:], in0=wf[:], scalar1=s_w[:], scalar2=MAGIC,
                op0=mybir.AluOpType.mult, op1=mybir.AluOpType.add,
            )
            # passB: wb = wf - MAGIC -> bf16 (small ints, exact)
            wb = wb_pool.tile([P, D], BF16, tag="wb", name=f"wb_{j}")
            if passb_engine == "vector":
                nc.vector.tensor_scalar_add(wb[:], wf[:], -MAGIC)
            else:
                nc.scalar.activation(
                    wb[:], wf[:], mybir.ActivationFunctionType.Identity,
                    bias=negmagic[:], scale=1.0,
                )
            # transpose via PE (avoids the serialized DMA-transpose chain),
            # then drain PSUM -> wqt with the {-1,0,1} clip fused in
            for q in range(n_dc // 4):
                ps4 = tp_pool.tile([P, 4, P], BF16, tag="tps", name=f"tps_{j}_{q}")
                for c in range(4):
                    dc = 4 * q + c
                    nc.tensor.transpose(
                        ps4[:, c, :], wb[:, ts(dc, P)], ident[:]
                    )
                nc.vector.tensor_scalar(
                    out=wqt[:, 4 * q : 4 * q + 4, ts(j, P)], in0=ps4[:],
                    scalar1=1.0, scalar2=-1.0,
                    op0=mybir.AluOpType.min, op1=mybir.AluOpType.max,
                )

        # group 0 W prep: passB on DVE so ACT only does the transposes
        for j in range(GJ):
            w_prep(j, passb_engine="vector")

        # ---------------- e-quant pipeline ----------------
        deq_tiles = {}
        qb_pair = [None]

        def e_quant(i):
            with tc.high_priority():
                _e_quant(i)

        def _e_quant(i):
            ef = ef_pool.tile([P, D], F32, tag="ef", name=f"ef_{i}")
            eld = nc.gpsimd.dma_start(out=ef[:], in_=e_d[ts(i, P), :])
            if i == 0:
                # e stream starts only once the W load phase is nearly done
                add_dep_helper(
                    eld.ins, w_load_ins[13].ins, sync=True,
                    reason="e loads after W loads",
                )
            rmax = scal_pool.tile([P, 1], F32, tag="rmax", name=f"rmax{i}", bufs=4)
            nc.vector.tensor_reduce(
                out=rmax[:], in_=ef[:], axis=mybir.AxisListType.X,
                op=mybir.AluOpType.max, apply_absolute_value=True,
            )
            rm_c = scal_pool.tile([P, 1], F32, tag="rmc", name=f"rmc{i}", bufs=4)
            nc.vector.tensor_scalar_max(rm_c[:], rmax[:], EPS)
            r0 = scal_pool.tile([P, 1], F32, tag="r0", name=f"r0_{i}", bufs=4)
            nc.vector.reciprocal(r0[:], rm_c[:])
            t1 = scal_pool.tile([P, 1], F32, tag="t1", name=f"t1_{i}", bufs=4)
            nc.vector.scalar_tensor_tensor(
                out=t1[:], in0=rm_c[:], scalar=-1.0, in1=r0[:],
                op0=mybir.AluOpType.mult, op1=mybir.AluOpType.mult,
            )
            nc.vector.tensor_scalar_add(t1[:], t1[:], 2.0)
            scale = scal_pool.tile([P, 1], F32, tag="scale", name=f"scale{i}", bufs=4)
            nc.vector.scalar_tensor_tensor(
                out=scale[:], in0=r0[:], scalar=127.0, in1=t1[:],
                op0=mybir.AluOpType.mult, op1=mybir.AluOpType.mult,
            )
            deq = scal_pool.tile([P, 1], F32, tag=f"deq{i}")
            nc.vector.tensor_scalar_mul(deq[:], rm_c[:], deqm[:])
            deq_tiles[i] = deq
            # passA in place on GpSimd (keeps DVE free for epilogues)
            nc.gpsimd.tensor_scalar(
                out=ef[:], in0=ef[:], scalar1=scale[:], scalar2=MAGIC,
                op0=mybir.AluOpType.mult, op1=mybir.AluOpType.add,
            )
            # passB: qb pair slot = ef - MAGIC -> bf16; one big transpose
            # per PAIR of tiles halves the serialized DMA-transpose chain
            if i % 2 == 0:
                qb_pair[0] = qb_pool.tile(
                    [P, 2, D], BF16, tag="qb", name=f"qb_{i}"
                )
            qb = qb_pair[0]
            nc.scalar.activation(
                qb[:, i % 2, :], ef[:], mybir.ActivationFunctionType.Identity,
                bias=negmagic[:], scale=1.0,
            )
            if i % 2 == 1:
                nc.scalar.dma_start_transpose(
                    out=eT[:, i - 1 : i + 1], in_=qb[:]
                )

        # ---------------- matmul blocks ----------------
        bo_tiles = {}

        def bo_load(g, i):
            bo_t = bo_pool.tile([P, MM_N], F32, tag="bo", name=f"bo_{g}_{i}")
            nc.sync.dma_start(out=bo_t[:], in_=bo_d[ts(i, P), ts(g, MM_N)])
            bo_tiles[(g, i)] = bo_t

        if with_h:
            h_d = io["h"]
            hf_tiles = {}

            def hf_load(g, i):
                hf = bo_pool.tile([P, MM_N], F32, tag="hf", name=f"hf_{g}_{i}")
                nc.sync.dma_start(out=hf[:], in_=h_d[ts(i, P), ts(g, MM_N)])
                hf_tiles[(g, i)] = hf

        def mm_block(g, i):
            bo_t = bo_tiles[(g, i)]
            ps = pp_pool.tile([P, MM_N], F32, tag="ps", name=f"ps_{g}_{i}")
            for d in range(n_dc):
                nc.tensor.matmul(
                    ps[:],
                    eT[:, i, d, :],
                    wqt[:, d, ts(g, MM_N)],
                    start=(d == 0),
                    stop=(d == n_dc - 1),
                )
            # epilogue: bo = psum*deq + bo   (fused dequant + add, in place)
            nc.vector.scalar_tensor_tensor(
                out=bo_t[:], in0=ps[:], scalar=deq_tiles[i][:], in1=bo_t[:],
                op0=mybir.AluOpType.mult, op1=mybir.AluOpType.add,
            )
            if with_h:
                hf = hf_tiles[(g, i)]
                nc.vector.tensor_tensor(
                    out=hf[:], in0=hf[:], in1=aeff[:, ts(g, MM_N)],
                    op=mybir.AluOpType.mult,
                )
                nc.vector.tensor_tensor(
                    out=bo_t[:], in0=bo_t[:], in1=hf[:], op=mybir.AluOpType.add,
                )
            # out store issued from ACT (idle in steady state)
            nc.scalar.dma_start(out=out_d[ts(i, P), ts(g, MM_N)], in_=bo_t[:])

        # ---------------- emission schedule ----------------
        half1 = list(range(n_tb // 2))
        half2 = list(range(n_tb // 2, n_tb))
        blocks = (
            [(0, i) for i in half1] + [(1, i) for i in half1]
            + [(0, i) for i in half2] + [(1, i) for i in half2]
            + [(2, i) for i in half1 + half2] + [(3, i) for i in half1 + half2]
        )
        BO_LAG = 6  # bo quarters prefetched this many blocks ahead

        # group-1 W reloads issue on sync right after the bo warm-up
        for g, i in blocks[:BO_LAG]:
            bo_load(g, i)
            if with_h:
                hf_load(g, i)
        for j in range(GJ, 2 * GJ):
            w_reload(j)

        for i in half1[:6]:
            e_quant(i)
        # group 1 W prep (reloads already landed)
        for j in range(GJ, 2 * GJ):
            w_prep(j, passb_engine="scalar")
        for i in half1[6:]:
            e_quant(i)

        emitted_e = n_tb // 2

        for k, (g, i) in enumerate(blocks):
            mm_block(g, i)
            if k + BO_LAG < len(blocks):
                ng, ni = blocks[k + BO_LAG]
                bo_load(ng, ni)
                if with_h:
                    hf_load(ng, ni)
            if k == 1 and emitted_e < n_tb:
                # second half of the e pipeline rides behind the first blocks
                for i2 in half2:
                    e_quant(i2)
                emitted_e = n_tb
            if k == 9:
                for j in range(2 * GJ, 3 * GJ):
                    w_reload(j)
            if k == 15:
                for j in range(2 * GJ, 3 * GJ):
                    w_prep(j, passb_engine="scalar")
            if k == 21:
                for j in range(3 * GJ, 4 * GJ):
                    w_reload(j)
            if k == 27:
                for j in range(3 * GJ, 4 * GJ):
                    w_prep(j, passb_engine="scalar")


def legalize_waits(nc):
    """Walrus in this container encodes at most ONE sync wait per ISA
    instruction (the 64B Events field) and refuses to split.  Rewrite any
    instruction carrying N>1 waits into N-1 single-wait NOP carrier
    instructions on the same engine placed immediately before it, keeping one
    wait on the original.  Waits are monotonic sem>=v conditions, so splitting
    preserves semantics exactly."""
    import bass_rust

    eng_map = {
        mybir.EngineType.SP: nc.sync,
        mybir.EngineType.DVE: nc.vector,
        mybir.EngineType.Activation: nc.scalar,
        mybir.EngineType.PE: nc.tensor,
        mybir.EngineType.Pool: nc.gpsimd,
    }
    for f in nc.m.functions:
        for blk in f.blocks:
            insts = list(blk.instructions)
            if not any(
                i.sync_info is not None and len(i.sync_info.on_wait) > 1
                for i in insts
            ):
                continue
            carriers = {}  # target inst name -> list of carrier insts
            for inst in insts:
                si = inst.sync_info
                if si is None or len(si.on_wait) <= 1:
                    continue
                waits = list(si.on_wait)
                cs = []
                for w in waits[:-1]:
                    bi = eng_map[inst.engine].nop(nofuse=True)
                    nop_inst = bi.ins
                    nop_inst.sync_info = bass_rust.SyncInfo(
                        on_wait=[w], on_update=[]
                    )
                    cs.append(nop_inst)
                carriers[inst.name] = cs
                inst.sync_info = bass_rust.SyncInfo(
                    on_wait=[waits[-1]], on_update=list(si.on_update)
                )
            carrier_names = {c.name for cs in carriers.values() for c in cs}
            for f2 in nc.m.functions:
                for blk2 in f2.blocks:
                    cur = list(blk2.instructions)
                    if any(i.name in carrier_names for i in cur):
                        blk2.instructions = [
                            i for i in cur if i.name not in carrier_names
                        ]
            new_list = []
            for inst in blk.instructions:
                for c in carriers.get(inst.name, ()):
                    new_list.append(c)
                new_list.append(inst)
            blk.instructions = new_list


def build_nc(Tc: int, D: int, with_h: bool):
    nc = bass.Bass("TRN2", target_bir_lowering=False, debug=False)
    io = {
        "e": nc.declare_dram_parameter("e", [Tc, D], F32, isOutput=False)[:],
        "bo": nc.declare_dram_parameter("bo", [Tc, D], F32, isOutput=False)[:],
        "w": nc.declare_dram_parameter("w", [D, D], F32, isOutput=False)[:],
    }
    if with_h:
        io["h"] = nc.declare_dram_parameter("h", [Tc, D], F32, isOutput=False)[:]
        io["a_raw"] = nc.declare_dram_parameter("a_raw", [1, D], F32, isOutput=False)[:]
    io["out"] = nc.declare_dram_parameter("out", [Tc, D], F32, isOutput=True)[:]
    with tile.TileContext(nc) as tc:
        build_kernel_body(tc, io, Tc, D, with_h)
    legalize_waits(nc)
    return nc


_NC_CACHE: dict = {}


def _get_nc(Tc: int, D: int, with_h: bool):
    key = (Tc, D, with_h)
    if key not in _NC_CACHE:
        _NC_CACHE[key] = build_nc(Tc, D, with_h)
    return _NC_CACHE[key]


def kernel(h, e, block_out, A_raw, W, _trace=False, _trace_kwargs=None):
    Bb, Tt, D = e.shape
    rows = Bb * Tt
    Tc = rows // N_CORES
    e2 = e.reshape(rows, D)
    bo2 = block_out.reshape(rows, D)
    h2 = h.reshape(rows, D)
    with_h = bool(np.any(A_raw))

    nc = _get_nc(Tc, D, with_h)
    in_maps = []
    for c in range(N_CORES):
        sl = slice(c * Tc, (c + 1) * Tc)
        m = {
            "e": np.ascontiguousarray(e2[sl]),
            "bo": np.ascontiguousarray(bo2[sl]),
            "w": np.ascontiguousarray(W),
        }
        if with_h:
            m["h"] = np.ascontiguousarray(h2[sl])
            m["a_raw"] = np.ascontiguousarray(A_raw.reshape(1, D))
        in_maps.append(m)

    res = run_bass_kernel_spmd(
        nc, in_maps, list(range(N_CORES)), trace=_trace,
        **(_trace_kwargs or {}),
    )
    out = np.concatenate([res.results[c]["out"] for c in range(N_CORES)], axis=0)
    if _trace:
        return out.reshape(Bb, Tt, D), res
    return out.reshape(Bb, Tt, D)


# revision 24
# speedup vs baseline: 1.0056x; 1.0056x over previous
"""Trainium2 Bass kernel for BitLTIInjection (BitNet-style fake-quantized linear
+ LTI injection):

    A_eff = 0.99*tanh(A_raw)
    e_q   = per-token absmax int8 fake quant of e
    W_q   = absmean ternary fake quant of W
    out   = A_eff*h + e_q @ W_q.T + block_out

Data-parallel over B*T across 8 cores; W replicated.  The quantized matmul
runs in bf16 (quantized values are small integers, so bf16 matmul with fp32
PSUM accumulation is numerically exact); dequant scales fold into the
PSUM->SBUF epilogue.  Rounding uses the f32 magic-number trick.

v2 schedule: W streams in first at full HBM rate while DVE computes the
absmean; ternarize+transpose runs per 4-row-tile output group, pipelined
against the matmul stream (group g's weights prep during group g-1's
matmuls).  Matmul blocks run in an order that gives the e-quant pipeline
runway: (g0,i0-7),(g1,i0-7),(g0,i8-15),(g1,i8-15),(g2,*),(g3,*).
Engine split: SP=loads, ACT=f32->bf16 passes + transposes, DVE=scale math +
epilogues, Pool=e reduces + W reloads + out stores, PE=matmuls.
"""

import numpy as np

import concourse.bass as bass
import concourse.mybir as mybir
import concourse.tile as tile
from concourse.bass import ts
from concourse.tile_rust import add_dep_helper
from concourse.bass_utils import run_bass_kernel_spmd

P = 128
MAGIC = 12582912.0  # 1.5 * 2**23: forces RNE-to-integer in f32
EPS = 1e-5
N_CORES = 8
F32 = mybir.dt.float32
BF16 = mybir.dt.bfloat16
MM_N = 512  # moving free dim per matmul (one PSUM bank of f32)


def build_kernel_body(tc: tile.TileContext, io: dict, Tc: int, D: int, with_h: bool):
    nc = tc.nc
    n_tb = Tc // P      # token blocks per core (16)
    n_dc = D // P       # contraction chunks (16)
    n_ob = D // MM_N    # output column groups (4)
    n_wt = D // P       # weight row tiles (16)
    GJ = n_wt // n_ob   # W row tiles per output group (4)

    e_d = io["e"]
    bo_d = io["bo"]
    w_d = io["w"]
    out_d = io["out"]

    resident_g0 = not with_h  # keep last GJ W f32 tiles resident -> no g0 reload
    wf_bufs = (GJ + 0) if resident_g0 else 2

    with (
        tc.tile_pool(name="wqt", bufs=1) as wqt_pool,
        tc.tile_pool(name="et", bufs=1) as et_pool,
        tc.tile_pool(name="wf", bufs=wf_bufs) as wf_pool,
        tc.tile_pool(name="wb", bufs=2) as wb_pool,
        tc.tile_pool(name="ef", bufs=2) as ef_pool,
        tc.tile_pool(name="qb", bufs=1) as qb_pool,
        tc.tile_pool(name="bo", bufs=5) as bo_pool,
        tc.tile_pool(name="scal", bufs=1) as scal_pool,
        tc.tile_pool(name="pp", bufs=6, space="PSUM") as pp_pool,
        tc.tile_pool(name="tp", bufs=2, space="PSUM") as tp_pool,
    ):
        # resident transposed ternary weights [d0, dc, o] and transposed e
        wqt = wqt_pool.tile([P, n_dc, D], BF16, tag="wqt")
        eT = et_pool.tile([P, n_tb, n_dc, P], BF16, tag="eT")

        ones_col = scal_pool.tile([P, 1], F32, tag="ones_col")
        nc.vector.memset(ones_col[:], 1.0)
        ones_row = scal_pool.tile([1, P], F32, tag="ones_row")
        nc.vector.memset(ones_row[:], 1.0)
        negmagic = scal_pool.tile([P, 1], F32, tag="negmagic")
        nc.vector.memset(negmagic[:], -MAGIC)
        # identity (bf16) for PE-transposes of the ternarized weights
        ident = scal_pool.tile([P, P], BF16, tag="ident")
        nc.vector.memset(ident[:], 1.0)
        nc.gpsimd.affine_select(
            ident[:], ident[:], pattern=[[1, P]],
            compare_op=mybir.AluOpType.is_equal, fill=0.0,
            base=0, channel_multiplier=-1,
        )

        # ---------------- W load (full-rate) + absmean ----------------
        # per-tile |W| row sums on DVE (idle during the load phase)
        parts = scal_pool.tile([P, n_wt], F32, tag="parts")
        if resident_g0:
            load_order = list(range(GJ, n_wt)) + list(range(GJ))
        else:
            load_order = list(range(n_wt))
        wf_tiles = {}
        w_load_ins = []
        for j in load_order:
            wf = wf_pool.tile([P, D], F32, tag="wf", name=f"wfm_{j}")
            w_load_ins.append(nc.sync.dma_start(out=wf[:], in_=w_d[ts(j, P), :]))
            nc.vector.tensor_reduce(
                out=parts[:, j : j + 1],
                in_=wf[:],
                axis=mybir.AxisListType.X,
                op=mybir.AluOpType.add,
                apply_absolute_value=True,
            )
            wf_tiles[j] = wf

        hp = tc.high_priority()
        hp.__enter__()
        acc = scal_pool.tile([P, 1], F32, tag="acc")
        nc.vector.tensor_reduce(
            out=acc[:], in_=parts[:], axis=mybir.AxisListType.X,
            op=mybir.AluOpType.add,
        )
        # cross-partition sum + broadcast via tiny PE matmuls
        tot_ps = pp_pool.tile([P, MM_N], F32, tag="ps", name="tot_ps")
        nc.tensor.matmul(tot_ps[:1, :1], ones_col[:], acc[:])
        tot_sb = scal_pool.tile([1, 1], F32, tag="tot_sb")
        nc.vector.tensor_copy(out=tot_sb[:], in_=tot_ps[:1, :1])
        asum_ps = pp_pool.tile([P, MM_N], F32, tag="ps", name="asum_ps")
        nc.tensor.matmul(asum_ps[:, :1], ones_row[:], tot_sb[:])
        allsum = scal_pool.tile([P, 1], F32, tag="allsum")
        nc.vector.tensor_copy(out=allsum[:], in_=asum_ps[:, :1])
        # m = max(mean_abs, EPS); s_w = 1/m (Newton-refined); deqm = m/127
        m_t = scal_pool.tile([P, 1], F32, tag="m_t")
        nc.vector.tensor_scalar(
            out=m_t[:], in0=allsum[:], scalar1=1.0 / (D * D), scalar2=EPS,
            op0=mybir.AluOpType.mult, op1=mybir.AluOpType.max,
        )
        r0w = scal_pool.tile([P, 1], F32, tag="r0w")
        nc.vector.reciprocal(r0w[:], m_t[:])
        t1w = scal_pool.tile([P, 1], F32, tag="t1w")
        nc.vector.scalar_tensor_tensor(
            out=t1w[:], in0=m_t[:], scalar=-1.0, in1=r0w[:],
            op0=mybir.AluOpType.mult, op1=mybir.AluOpType.mult,
        )
        nc.vector.tensor_scalar_add(t1w[:], t1w[:], 2.0)
        s_w = scal_pool.tile([P, 1], F32, tag="s_w")
        nc.vector.tensor_scalar_mul(s_w[:], r0w[:], t1w[:])
        deqm = scal_pool.tile([P, 1], F32, tag="deqm")
        nc.vector.tensor_scalar_mul(deqm[:], m_t[:], 1.0 / 127.0)
        hp.__exit__(None, None, None)

        # ---------------- A_eff (only if nonzero A_raw) ----------------
        if with_h:
            a_d = io["a_raw"]
            a1 = scal_pool.tile([1, D], F32, tag="a1")
            nc.sync.dma_start(out=a1[:], in_=a_d[:, :])
            aeff = scal_pool.tile([P, D], F32, tag="aeff")
            for ob in range(n_ob):
                ab_ps = pp_pool.tile([P, MM_N], F32, tag="ps", name=f"ab_ps{ob}")
                nc.tensor.matmul(ab_ps[:], ones_row[:], a1[:, ts(ob, MM_N)])
                nc.vector.tensor_copy(out=aeff[:, ts(ob, MM_N)], in_=ab_ps[:])
            nc.scalar.activation(
                aeff[:], aeff[:], mybir.ActivationFunctionType.Tanh
            )
            nc.vector.tensor_scalar_mul(aeff[:], aeff[:], 0.99)

        # ---------------- W ternarize helpers ----------------
        def w_reload(j):
            wf = wf_pool.tile([P, D], F32, tag="wf", name=f"wfr_{j}")
            nc.sync.dma_start(out=wf[:], in_=w_d[ts(j, P), :])
            wf_tiles[j] = wf

        def w_prep(j, passb_engine):
            """round(W*s_w) -> bf16, transpose into wqt, clip later."""
            wf = wf_tiles[j]
            # passA in place: wf = wf*s_w + MAGIC  (rounds to int, biased)
            nc.vector.tensor_scalar(
                out=wf[:], in0=wf[:], scalar1=s_w[:], scalar2=MAGIC,
                op0=mybir.AluOpType.mult, op1=mybir.AluOpType.add,
            )
            # passB: wb = wf - MAGIC -> bf16 (small ints, exact)
            wb = wb_pool.tile([P, D], BF16, tag="wb", name=f"wb_{j}")
            if passb_engine == "vector":
                nc.vector.tensor_scalar_add(wb[:], wf[:], -MAGIC)
            else:
                nc.scalar.activation(
                    wb[:], wf[:], mybir.ActivationFunctionType.Identity,
                    bias=negmagic[:], scale=1.0,
                )
            # transpose via PE (avoids the serialized DMA-transpose chain),
            # then drain PSUM -> wqt with the {-1,0,1} clip fused in
            for q in range(n_dc // 4):
                ps4 = tp_pool.tile([P, 4, P], BF16, tag="tps", name=f"tps_{j}_{q}")
                for c in range(4):
                    dc = 4 * q + c
                    nc.tensor.transpose(
                        ps4[:, c, :], wb[:, ts(dc, P)], ident[:]
                    )
                nc.vector.tensor_scalar(
                    out=wqt[:, 4 * q : 4 * q + 4, ts(j, P)], in0=ps4[:],
                    scalar1=1.0, scalar2=-1.0,
                    op0=mybir.AluOpType.min, op1=mybir.AluOpType.max,
                )

        # group 0 W prep: passB on DVE so ACT only does the transposes
        for j in range(GJ):
            w_prep(j, passb_engine="vector")

        # ---------------- e-quant pipeline ----------------
        deq_tiles = {}
        qb_pair = [None]

        def e_quant(i):
            ef = ef_pool.tile([P, D], F32, tag="ef", name=f"ef_{i}")
            eld = nc.gpsimd.dma_start(out=ef[:], in_=e_d[ts(i, P), :])
            if i == 0:
                # e stream starts only once the W load phase is nearly done
                add_dep_helper(
                    eld.ins, w_load_ins[13].ins, sync=True,
                    reason="e loads after W loads",
                )
            rmax = scal_pool.tile([P, 1], F32, tag="rmax", name=f"rmax{i}", bufs=4)
            nc.vector.tensor_reduce(
                out=rmax[:], in_=ef[:], axis=mybir.AxisListType.X,
                op=mybir.AluOpType.max, apply_absolute_value=True,
            )
            rm_c = scal_pool.tile([P, 1], F32, tag="rmc", name=f"rmc{i}", bufs=4)
            nc.vector.tensor_scalar_max(rm_c[:], rmax[:], EPS)
            r0 = scal_pool.tile([P, 1], F32, tag="r0", name=f"r0_{i}", bufs=4)
            nc.vector.reciprocal(r0[:], rm_c[:])
            t1 = scal_pool.tile([P, 1], F32, tag="t1", name=f"t1_{i}", bufs=4)
            nc.vector.scalar_tensor_tensor(
                out=t1[:], in0=rm_c[:], scalar=-1.0, in1=r0[:],
                op0=mybir.AluOpType.mult, op1=mybir.AluOpType.mult,
            )
            nc.vector.tensor_scalar_add(t1[:], t1[:], 2.0)
            scale = scal_pool.tile([P, 1], F32, tag="scale", name=f"scale{i}", bufs=4)
            nc.vector.scalar_tensor_tensor(
                out=scale[:], in0=r0[:], scalar=127.0, in1=t1[:],
                op0=mybir.AluOpType.mult, op1=mybir.AluOpType.mult,
            )
            deq = scal_pool.tile([P, 1], F32, tag=f"deq{i}")
            nc.vector.tensor_scalar_mul(deq[:], rm_c[:], deqm[:])
            deq_tiles[i] = deq
            # passA in place on GpSimd (keeps DVE free for epilogues)
            nc.gpsimd.tensor_scalar(
                out=ef[:], in0=ef[:], scalar1=scale[:], scalar2=MAGIC,
                op0=mybir.AluOpType.mult, op1=mybir.AluOpType.add,
            )
            # passB: qb pair slot = ef - MAGIC -> bf16; one big transpose
            # per PAIR of tiles halves the serialized DMA-transpose chain
            if i % 2 == 0:
                qb_pair[0] = qb_pool.tile(
                    [P, 2, D], BF16, tag="qb", name=f"qb_{i}"
                )
            qb = qb_pair[0]
            nc.scalar.activation(
                qb[:, i % 2, :], ef[:], mybir.ActivationFunctionType.Identity,
                bias=negmagic[:], scale=1.0,
            )
            if i % 2 == 1:
                nc.scalar.dma_start_transpose(
                    out=eT[:, i - 1 : i + 1], in_=qb[:]
                )

        # ---------------- matmul blocks ----------------
        bo_tiles = {}

        def bo_load(g, i):
            bo_t = bo_pool.tile([P, MM_N], F32, tag="bo", name=f"bo_{g}_{i}")
            nc.sync.dma_start(out=bo_t[:], in_=bo_d[ts(i, P), ts(g, MM_N)])
            bo_tiles[(g, i)] = bo_t

        if with_h:
            h_d = io["h"]
            hf_tiles = {}

            def hf_load(g, i):
                hf = bo_pool.tile([P, MM_N], F32, tag="hf", name=f"hf_{g}_{i}")
                nc.sync.dma_start(out=hf[:], in_=h_d[ts(i, P), ts(g, MM_N)])
                hf_tiles[(g, i)] = hf

        def mm_block(g, i):
            bo_t = bo_tiles[(g, i)]
            ps = pp_pool.tile([P, MM_N], F32, tag="ps", name=f"ps_{g}_{i}")
            for d in range(n_dc):
                nc.tensor.matmul(
                    ps[:],
                    eT[:, i, d, :],
                    wqt[:, d, ts(g, MM_N)],
                    start=(d == 0),
                    stop=(d == n_dc - 1),
                )
            # epilogue: bo = psum*deq + bo   (fused dequant + add, in place)
            nc.vector.scalar_tensor_tensor(
                out=bo_t[:], in0=ps[:], scalar=deq_tiles[i][:], in1=bo_t[:],
                op0=mybir.AluOpType.mult, op1=mybir.AluOpType.add,
            )
            if with_h:
                hf = hf_tiles[(g, i)]
                nc.vector.tensor_tensor(
                    out=hf[:], in0=hf[:], in1=aeff[:, ts(g, MM_N)],
                    op=mybir.AluOpType.mult,
                )
                nc.vector.tensor_tensor(
                    out=bo_t[:], in0=bo_t[:], in1=hf[:], op=mybir.AluOpType.add,
                )
            # out store issued from ACT (idle in steady state)
            nc.scalar.dma_start(out=out_d[ts(i, P), ts(g, MM_N)], in_=bo_t[:])

        # ---------------- emission schedule ----------------
        half1 = list(range(n_tb // 2))
        half2 = list(range(n_tb // 2, n_tb))
        blocks = (
            [(0, i) for i in half1] + [(1, i) for i in half1]
            + [(0, i) for i in half2] + [(1, i) for i in half2]
            + [(2, i) for i in half1 + half2] + [(3, i) for i in half1 + half2]
        )
        BO_LAG = 6  # bo quarters prefetched this many blocks ahead

        # group-1 W reloads issue on sync right after the bo warm-up
        for g, i in blocks[:BO_LAG]:
            bo_load(g, i)
            if with_h:
                hf_load(g, i)
        for j in range(GJ, 2 * GJ):
            w_reload(j)

        for i in half1[:6]:
            e_quant(i)
        # group 1 W prep (reloads already landed)
        for j in range(GJ, 2 * GJ):
            w_prep(j, passb_engine="scalar")
        for i in half1[6:]:
            e_quant(i)

        emitted_e = n_tb // 2

        for k, (g, i) in enumerate(blocks):
            mm_block(g, i)
            if k + BO_LAG < len(blocks):
                ng, ni = blocks[k + BO_LAG]
                bo_load(ng, ni)
                if with_h:
                    hf_load(ng, ni)
            if k == 1 and emitted_e < n_tb:
                # second half of the e pipeline rides behind the first blocks
                for i2 in half2:
                    e_quant(i2)
                emitted_e = n_tb
            if k == 9:
                for j in range(2 * GJ, 3 * GJ):
                    w_reload(j)
            if k == 15:
                for j in range(2 * GJ, 3 * GJ):
                    w_prep(j, passb_engine="scalar")
            if k == 21:
                for j in range(3 * GJ, 4 * GJ):
                    w_reload(j)
            if k == 27:
                for j in range(3 * GJ, 4 * GJ):
                    w_prep(j, passb_engine="scalar")


def legalize_waits(nc):
    """Walrus in this container encodes at most ONE sync wait per ISA
    instruction (the 64B Events field) and refuses to split.  Rewrite any
    instruction carrying N>1 waits into N-1 single-wait NOP carrier
    instructions on the same engine placed immediately before it, keeping one
    wait on the original.  Waits are monotonic sem>=v conditions, so splitting
    preserves semantics exactly."""
    import bass_rust

    eng_map = {
        mybir.EngineType.SP: nc.sync,
        mybir.EngineType.DVE: nc.vector,
        mybir.EngineType.Activation: nc.scalar,
        mybir.EngineType.PE: nc.tensor,
        mybir.EngineType.Pool: nc.gpsimd,
    }
    for f in nc.m.functions:
        for blk in f.blocks:
            insts = list(blk.instructions)
            if not any(
                i.sync_info is not None and len(i.sync_info.on_wait) > 1
                for i in insts
            ):
                continue
            carriers = {}  # target inst name -> list of carrier insts
            for inst in insts:
                si = inst.sync_info
                if si is None or len(si.on_wait) <= 1:
                    continue
                waits = list(si.on_wait)
                cs = []
                for w in waits[:-1]:
                    bi = eng_map[inst.engine].nop(nofuse=True)
                    nop_inst = bi.ins
                    nop_inst.sync_info = bass_rust.SyncInfo(
                        on_wait=[w], on_update=[]
                    )
                    cs.append(nop_inst)
                carriers[inst.name] = cs
                inst.sync_info = bass_rust.SyncInfo(
                    on_wait=[waits[-1]], on_update=list(si.on_update)
                )
            carrier_names = {c.name for cs in carriers.values() for c in cs}
            for f2 in nc.m.functions:
                for blk2 in f2.blocks:
                    cur = list(blk2.instructions)
                    if any(i.name in carrier_names for i in cur):
                        blk2.instructions = [
                            i for i in cur if i.name not in carrier_names
                        ]
            new_list = []
            for inst in blk.instructions:
                for c in carriers.get(inst.name, ()):
                    new_list.append(c)
                new_list.append(inst)
            blk.instructions = new_list


def build_nc(Tc: int, D: int, with_h: bool):
    nc = bass.Bass("TRN2", target_bir_lowering=False, debug=False)
    io = {
        "e": nc.declare_dram_parameter("e", [Tc, D], F32, isOutput=False)[:],
        "bo": nc.declare_dram_parameter("bo", [Tc, D], F32, isOutput=False)[:],
        "w": nc.declare_dram_parameter("w", [D, D], F32, isOutput=False)[:],
    }
    if with_h:
        io["h"] = nc.declare_dram_parameter("h", [Tc, D], F32, isOutput=False)[:]
        io["a_raw"] = nc.declare_dram_parameter("a_raw", [1, D], F32, isOutput=False)[:]
    io["out"] = nc.declare_dram_parameter("out", [Tc, D], F32, isOutput=True)[:]
    with tile.TileContext(nc) as tc:
        build_kernel_body(tc, io, Tc, D, with_h)
    legalize_waits(nc)
    return nc


_NC_CACHE: dict = {}


def _get_nc(Tc: int, D: int, with_h: bool):
    key = (Tc, D, with_h)
    if key not in _NC_CACHE:
        _NC_CACHE[key] = build_nc(Tc, D, with_h)
    return _NC_CACHE[key]


def kernel(h, e, block_out, A_raw, W, _trace=False, _trace_kwargs=None):
    Bb, Tt, D = e.shape
    rows = Bb * Tt
    Tc = rows // N_CORES
    e2 = e.reshape(rows, D)
    bo2 = block_out.reshape(rows, D)
    h2 = h.reshape(rows, D)
    with_h = bool(np.any(A_raw))

    nc = _get_nc(Tc, D, with_h)
    in_maps = []
    for c in range(N_CORES):
        sl = slice(c * Tc, (c + 1) * Tc)
        m = {
            "e": np.ascontiguousarray(e2[sl]),
            "bo": np.ascontiguousarray(bo2[sl]),
            "w": np.ascontiguousarray(W),
        }
        if with_h:
            m["h"] = np.ascontiguousarray(h2[sl])
            m["a_raw"] = np.ascontiguousarray(A_raw.reshape(1, D))
        in_maps.append(m)

    res = run_bass_kernel_spmd(
        nc, in_maps, list(range(N_CORES)), trace=_trace,
        **(_trace_kwargs or {}),
    )
    out = np.concatenate([res.results[c]["out"] for c in range(N_CORES)], axis=0)
    if _trace:
        return out.reshape(Bb, Tt, D), res
    return out.reshape(Bb, Tt, D)


# revision 25
# speedup vs baseline: 1.0803x; 1.0742x over previous
"""Trainium2 Bass kernel for BitLTIInjection (BitNet-style fake-quantized linear
+ LTI injection):

    A_eff = 0.99*tanh(A_raw)
    e_q   = per-token absmax int8 fake quant of e
    W_q   = absmean ternary fake quant of W
    out   = A_eff*h + e_q @ W_q.T + block_out

Data-parallel over B*T across 8 cores; W replicated.  The quantized matmul
runs in bf16 (quantized values are small integers, so bf16 matmul with fp32
PSUM accumulation is numerically exact); dequant scales fold into the
PSUM->SBUF epilogue.  Rounding uses the f32 magic-number trick.

v2 schedule: W streams in first at full HBM rate while DVE computes the
absmean; ternarize+transpose runs per 4-row-tile output group, pipelined
against the matmul stream (group g's weights prep during group g-1's
matmuls).  Matmul blocks run in an order that gives the e-quant pipeline
runway: (g0,i0-7),(g1,i0-7),(g0,i8-15),(g1,i8-15),(g2,*),(g3,*).
Engine split: SP=loads, ACT=f32->bf16 passes + transposes, DVE=scale math +
epilogues, Pool=e reduces + W reloads + out stores, PE=matmuls.
"""

import numpy as np

import concourse.bass as bass
import concourse.mybir as mybir
import concourse.tile as tile
from concourse.bass import ts
from concourse.tile_rust import add_dep_helper
from concourse.bass_utils import run_bass_kernel_spmd

P = 128
MAGIC = 12582912.0  # 1.5 * 2**23: forces RNE-to-integer in f32
EPS = 1e-5
N_CORES = 8
F32 = mybir.dt.float32
BF16 = mybir.dt.bfloat16
MM_N = 512  # moving free dim per matmul (one PSUM bank of f32)


def build_kernel_body(tc: tile.TileContext, io: dict, Tc: int, D: int, with_h: bool):
    nc = tc.nc
    n_tb = Tc // P      # token blocks per core (16)
    n_dc = D // P       # contraction chunks (16)
    n_ob = D // MM_N    # output column groups (4)
    n_wt = D // P       # weight row tiles (16)
    GJ = n_wt // n_ob   # W row tiles per output group (4)

    e_d = io["e"]
    bo_d = io["bo"]
    w_d = io["w"]
    out_d = io["out"]

    resident_g0 = not with_h  # keep last GJ W f32 tiles resident -> no g0 reload
    wf_bufs = (GJ + 0) if resident_g0 else 2

    with (
        tc.tile_pool(name="wqt", bufs=1) as wqt_pool,
        tc.tile_pool(name="et", bufs=1) as et_pool,
        tc.tile_pool(name="wf", bufs=wf_bufs) as wf_pool,
        tc.tile_pool(name="wb", bufs=2) as wb_pool,
        tc.tile_pool(name="ef", bufs=2) as ef_pool,
        tc.tile_pool(name="qb", bufs=1) as qb_pool,
        tc.tile_pool(name="bo", bufs=5) as bo_pool,
        tc.tile_pool(name="scal", bufs=1) as scal_pool,
        tc.tile_pool(name="pp", bufs=6, space="PSUM") as pp_pool,
        tc.tile_pool(name="tp", bufs=2, space="PSUM") as tp_pool,
    ):
        # resident transposed ternary weights [d0, dc, o] and transposed e
        wqt = wqt_pool.tile([P, n_dc, D], BF16, tag="wqt")
        eT = et_pool.tile([P, n_tb, n_dc, P], BF16, tag="eT")

        ones_col = scal_pool.tile([P, 1], F32, tag="ones_col")
        nc.vector.memset(ones_col[:], 1.0)
        ones_row = scal_pool.tile([1, P], F32, tag="ones_row")
        nc.vector.memset(ones_row[:], 1.0)
        negmagic = scal_pool.tile([P, 1], F32, tag="negmagic")
        nc.vector.memset(negmagic[:], -MAGIC)
        # identity (bf16) for PE-transposes of the ternarized weights
        ident = scal_pool.tile([P, P], BF16, tag="ident")
        nc.vector.memset(ident[:], 1.0)
        nc.gpsimd.affine_select(
            ident[:], ident[:], pattern=[[1, P]],
            compare_op=mybir.AluOpType.is_equal, fill=0.0,
            base=0, channel_multiplier=-1,
        )

        # ---------------- W load (full-rate) + absmean ----------------
        # per-tile |W| row sums on DVE (idle during the load phase)
        parts = scal_pool.tile([P, n_wt], F32, tag="parts")
        if resident_g0:
            load_order = list(range(GJ, n_wt)) + list(range(GJ))
        else:
            load_order = list(range(n_wt))
        wf_tiles = {}
        w_load_ins = []
        for j in load_order:
            wf = wf_pool.tile([P, D], F32, tag="wf", name=f"wfm_{j}")
            w_load_ins.append(nc.sync.dma_start(out=wf[:], in_=w_d[ts(j, P), :]))
            nc.vector.tensor_reduce(
                out=parts[:, j : j + 1],
                in_=wf[:],
                axis=mybir.AxisListType.X,
                op=mybir.AluOpType.add,
                apply_absolute_value=True,
            )
            wf_tiles[j] = wf

        hp = tc.high_priority()
        hp.__enter__()
        acc = scal_pool.tile([P, 1], F32, tag="acc")
        nc.vector.tensor_reduce(
            out=acc[:], in_=parts[:], axis=mybir.AxisListType.X,
            op=mybir.AluOpType.add,
        )
        # cross-partition sum + broadcast via tiny PE matmuls
        tot_ps = pp_pool.tile([P, MM_N], F32, tag="ps", name="tot_ps")
        nc.tensor.matmul(tot_ps[:1, :1], ones_col[:], acc[:])
        tot_sb = scal_pool.tile([1, 1], F32, tag="tot_sb")
        nc.vector.tensor_copy(out=tot_sb[:], in_=tot_ps[:1, :1])
        asum_ps = pp_pool.tile([P, MM_N], F32, tag="ps", name="asum_ps")
        nc.tensor.matmul(asum_ps[:, :1], ones_row[:], tot_sb[:])
        allsum = scal_pool.tile([P, 1], F32, tag="allsum")
        nc.vector.tensor_copy(out=allsum[:], in_=asum_ps[:, :1])
        # m = max(mean_abs, EPS); s_w = 1/m (Newton-refined); deqm = m/127
        m_t = scal_pool.tile([P, 1], F32, tag="m_t")
        nc.vector.tensor_scalar(
            out=m_t[:], in0=allsum[:], scalar1=1.0 / (D * D), scalar2=EPS,
            op0=mybir.AluOpType.mult, op1=mybir.AluOpType.max,
        )
        r0w = scal_pool.tile([P, 1], F32, tag="r0w")
        nc.vector.reciprocal(r0w[:], m_t[:])
        t1w = scal_pool.tile([P, 1], F32, tag="t1w")
        nc.vector.scalar_tensor_tensor(
            out=t1w[:], in0=m_t[:], scalar=-1.0, in1=r0w[:],
            op0=mybir.AluOpType.mult, op1=mybir.AluOpType.mult,
        )
        nc.vector.tensor_scalar_add(t1w[:], t1w[:], 2.0)
        s_w = scal_pool.tile([P, 1], F32, tag="s_w")
        nc.vector.tensor_scalar_mul(s_w[:], r0w[:], t1w[:])
        deqm = scal_pool.tile([P, 1], F32, tag="deqm")
        nc.vector.tensor_scalar_mul(deqm[:], m_t[:], 1.0 / 127.0)
        hp.__exit__(None, None, None)

        # ---------------- A_eff (only if nonzero A_raw) ----------------
        if with_h:
            a_d = io["a_raw"]
            a1 = scal_pool.tile([1, D], F32, tag="a1")
            nc.sync.dma_start(out=a1[:], in_=a_d[:, :])
            aeff = scal_pool.tile([P, D], F32, tag="aeff")
            for ob in range(n_ob):
                ab_ps = pp_pool.tile([P, MM_N], F32, tag="ps", name=f"ab_ps{ob}")
                nc.tensor.matmul(ab_ps[:], ones_row[:], a1[:, ts(ob, MM_N)])
                nc.vector.tensor_copy(out=aeff[:, ts(ob, MM_N)], in_=ab_ps[:])
            nc.scalar.activation(
                aeff[:], aeff[:], mybir.ActivationFunctionType.Tanh
            )
            nc.vector.tensor_scalar_mul(aeff[:], aeff[:], 0.99)

        # ---------------- W ternarize helpers ----------------
        def w_reload(j):
            wf = wf_pool.tile([P, D], F32, tag="wf", name=f"wfr_{j}")
            nc.sync.dma_start(out=wf[:], in_=w_d[ts(j, P), :])
            wf_tiles[j] = wf

        def w_prep(j, passb_engine):
            """round(W*s_w) -> bf16, transpose into wqt, clip later."""
            wf = wf_tiles[j]
            # passA in place: wf = wf*s_w + MAGIC  (rounds to int, biased)
            nc.vector.tensor_scalar(
                out=wf[:], in0=wf[:], scalar1=s_w[:], scalar2=MAGIC,
                op0=mybir.AluOpType.mult, op1=mybir.AluOpType.add,
            )
            # passB: wb = wf - MAGIC -> bf16 (small ints, exact)
            wb = wb_pool.tile([P, D], BF16, tag="wb", name=f"wb_{j}")
            if passb_engine == "vector":
                nc.vector.tensor_scalar_add(wb[:], wf[:], -MAGIC)
            else:
                nc.scalar.activation(
                    wb[:], wf[:], mybir.ActivationFunctionType.Identity,
                    bias=negmagic[:], scale=1.0,
                )
            # transpose via PE (avoids the serialized DMA-transpose chain),
            # then drain PSUM -> wqt with the {-1,0,1} clip fused in
            for q in range(n_dc // 4):
                ps4 = tp_pool.tile([P, 4, P], BF16, tag="tps", name=f"tps_{j}_{q}")
                for c in range(4):
                    dc = 4 * q + c
                    nc.tensor.transpose(
                        ps4[:, c, :], wb[:, ts(dc, P)], ident[:]
                    )
                nc.vector.tensor_scalar(
                    out=wqt[:, 4 * q : 4 * q + 4, ts(j, P)], in0=ps4[:],
                    scalar1=1.0, scalar2=-1.0,
                    op0=mybir.AluOpType.min, op1=mybir.AluOpType.max,
                )

        # group 0 W prep: passB on DVE so ACT only does the transposes
        for j in range(GJ):
            w_prep(j, passb_engine="vector")

        # ---------------- e-quant pipeline ----------------
        deq_tiles = {}
        qb_pair = [None]

        def e_quant(i):
            ef = ef_pool.tile([P, D], F32, tag="ef", name=f"ef_{i}")
            eld = nc.gpsimd.dma_start(out=ef[:], in_=e_d[ts(i, P), :])
            if i == 0:
                # e stream starts only once the W load phase is nearly done
                add_dep_helper(
                    eld.ins, w_load_ins[6].ins, sync=True,
                    reason="e loads after W loads",
                )
            rmax = scal_pool.tile([P, 1], F32, tag="rmax", name=f"rmax{i}", bufs=4)
            nc.vector.tensor_reduce(
                out=rmax[:], in_=ef[:], axis=mybir.AxisListType.X,
                op=mybir.AluOpType.max, apply_absolute_value=True,
            )
            rm_c = scal_pool.tile([P, 1], F32, tag="rmc", name=f"rmc{i}", bufs=4)
            nc.vector.tensor_scalar_max(rm_c[:], rmax[:], EPS)
            r0 = scal_pool.tile([P, 1], F32, tag="r0", name=f"r0_{i}", bufs=4)
            nc.vector.reciprocal(r0[:], rm_c[:])
            t1 = scal_pool.tile([P, 1], F32, tag="t1", name=f"t1_{i}", bufs=4)
            nc.vector.scalar_tensor_tensor(
                out=t1[:], in0=rm_c[:], scalar=-1.0, in1=r0[:],
                op0=mybir.AluOpType.mult, op1=mybir.AluOpType.mult,
            )
            nc.vector.tensor_scalar_add(t1[:], t1[:], 2.0)
            scale = scal_pool.tile([P, 1], F32, tag="scale", name=f"scale{i}", bufs=4)
            nc.vector.scalar_tensor_tensor(
                out=scale[:], in0=r0[:], scalar=127.0, in1=t1[:],
                op0=mybir.AluOpType.mult, op1=mybir.AluOpType.mult,
            )
            deq = scal_pool.tile([P, 1], F32, tag=f"deq{i}")
            nc.vector.tensor_scalar_mul(deq[:], rm_c[:], deqm[:])
            deq_tiles[i] = deq
            # passA in place on GpSimd (keeps DVE free for epilogues)
            nc.gpsimd.tensor_scalar(
                out=ef[:], in0=ef[:], scalar1=scale[:], scalar2=MAGIC,
                op0=mybir.AluOpType.mult, op1=mybir.AluOpType.add,
            )
            # passB: qb pair slot = ef - MAGIC -> bf16; one big transpose
            # per PAIR of tiles halves the serialized DMA-transpose chain
            if i % 2 == 0:
                qb_pair[0] = qb_pool.tile(
                    [P, 2, D], BF16, tag="qb", name=f"qb_{i}"
                )
            qb = qb_pair[0]
            nc.scalar.activation(
                qb[:, i % 2, :], ef[:], mybir.ActivationFunctionType.Identity,
                bias=negmagic[:], scale=1.0,
            )
            if i % 2 == 1:
                nc.scalar.dma_start_transpose(
                    out=eT[:, i - 1 : i + 1], in_=qb[:]
                )

        # ---------------- matmul blocks ----------------
        bo_tiles = {}

        def bo_load(g, i):
            bo_t = bo_pool.tile([P, MM_N], F32, tag="bo", name=f"bo_{g}_{i}")
            nc.sync.dma_start(out=bo_t[:], in_=bo_d[ts(i, P), ts(g, MM_N)])
            bo_tiles[(g, i)] = bo_t

        if with_h:
            h_d = io["h"]
            hf_tiles = {}

            def hf_load(g, i):
                hf = bo_pool.tile([P, MM_N], F32, tag="hf", name=f"hf_{g}_{i}")
                nc.sync.dma_start(out=hf[:], in_=h_d[ts(i, P), ts(g, MM_N)])
                hf_tiles[(g, i)] = hf

        def mm_block(g, i):
            bo_t = bo_tiles[(g, i)]
            ps = pp_pool.tile([P, MM_N], F32, tag="ps", name=f"ps_{g}_{i}")
            for d in range(n_dc):
                nc.tensor.matmul(
                    ps[:],
                    eT[:, i, d, :],
                    wqt[:, d, ts(g, MM_N)],
                    start=(d == 0),
                    stop=(d == n_dc - 1),
                )
            # epilogue: bo = psum*deq + bo   (fused dequant + add, in place)
            nc.vector.scalar_tensor_tensor(
                out=bo_t[:], in0=ps[:], scalar=deq_tiles[i][:], in1=bo_t[:],
                op0=mybir.AluOpType.mult, op1=mybir.AluOpType.add,
            )
            if with_h:
                hf = hf_tiles[(g, i)]
                nc.vector.tensor_tensor(
                    out=hf[:], in0=hf[:], in1=aeff[:, ts(g, MM_N)],
                    op=mybir.AluOpType.mult,
                )
                nc.vector.tensor_tensor(
                    out=bo_t[:], in0=bo_t[:], in1=hf[:], op=mybir.AluOpType.add,
                )
            # out store issued from ACT (idle in steady state)
            nc.scalar.dma_start(out=out_d[ts(i, P), ts(g, MM_N)], in_=bo_t[:])

        # ---------------- emission schedule ----------------
        half1 = list(range(n_tb // 2))
        half2 = list(range(n_tb // 2, n_tb))
        blocks = (
            [(0, i) for i in half1] + [(1, i) for i in half1]
            + [(0, i) for i in half2] + [(1, i) for i in half2]
            + [(2, i) for i in half1 + half2] + [(3, i) for i in half1 + half2]
        )
        BO_LAG = 6  # bo quarters prefetched this many blocks ahead

        # group-1 W reloads issue on sync right after the bo warm-up
        for g, i in blocks[:BO_LAG]:
            bo_load(g, i)
            if with_h:
                hf_load(g, i)
        for j in range(GJ, 2 * GJ):
            w_reload(j)

        for i in half1[:6]:
            e_quant(i)
        # group 1 W prep (reloads already landed)
        for j in range(GJ, 2 * GJ):
            w_prep(j, passb_engine="scalar")
        for i in half1[6:]:
            e_quant(i)

        emitted_e = n_tb // 2

        for k, (g, i) in enumerate(blocks):
            mm_block(g, i)
            if k + BO_LAG < len(blocks):
                ng, ni = blocks[k + BO_LAG]
                bo_load(ng, ni)
                if with_h:
                    hf_load(ng, ni)
            if k == 1 and emitted_e < n_tb:
                # second half of the e pipeline rides behind the first blocks
                for i2 in half2:
                    e_quant(i2)
                emitted_e = n_tb
            if k == 7:
                for j in range(2 * GJ, 3 * GJ):
                    w_reload(j)
            if k == 11:
                for j in range(2 * GJ, 3 * GJ):
                    w_prep(j, passb_engine="scalar")
            if k == 17:
                for j in range(3 * GJ, 4 * GJ):
                    w_reload(j)
            if k == 23:
                for j in range(3 * GJ, 4 * GJ):
                    w_prep(j, passb_engine="scalar")


def legalize_waits(nc):
    """Walrus in this container encodes at most ONE sync wait per ISA
    instruction (the 64B Events field) and refuses to split.  Rewrite any
    instruction carrying N>1 waits into N-1 single-wait NOP carrier
    instructions on the same engine placed immediately before it, keeping one
    wait on the original.  Waits are monotonic sem>=v conditions, so splitting
    preserves semantics exactly."""
    import bass_rust

    eng_map = {
        mybir.EngineType.SP: nc.sync,
        mybir.EngineType.DVE: nc.vector,
        mybir.EngineType.Activation: nc.scalar,
        mybir.EngineType.PE: nc.tensor,
        mybir.EngineType.Pool: nc.gpsimd,
    }
    for f in nc.m.functions:
        for blk in f.blocks:
            insts = list(blk.instructions)
            if not any(
                i.sync_info is not None and len(i.sync_info.on_wait) > 1
                for i in insts
            ):
                continue
            carriers = {}  # target inst name -> list of carrier insts
            for inst in insts:
                si = inst.sync_info
                if si is None or len(si.on_wait) <= 1:
                    continue
                waits = list(si.on_wait)
                cs = []
                for w in waits[:-1]:
                    bi = eng_map[inst.engine].nop(nofuse=True)
                    nop_inst = bi.ins
                    nop_inst.sync_info = bass_rust.SyncInfo(
                        on_wait=[w], on_update=[]
                    )
                    cs.append(nop_inst)
                carriers[inst.name] = cs
                inst.sync_info = bass_rust.SyncInfo(
                    on_wait=[waits[-1]], on_update=list(si.on_update)
                )
            carrier_names = {c.name for cs in carriers.values() for c in cs}
            for f2 in nc.m.functions:
                for blk2 in f2.blocks:
                    cur = list(blk2.instructions)
                    if any(i.name in carrier_names for i in cur):
                        blk2.instructions = [
                            i for i in cur if i.name not in carrier_names
                        ]
            new_list = []
            for inst in blk.instructions:
                for c in carriers.get(inst.name, ()):
                    new_list.append(c)
                new_list.append(inst)
            blk.instructions = new_list


def build_nc(Tc: int, D: int, with_h: bool):
    nc = bass.Bass("TRN2", target_bir_lowering=False, debug=False)
    io = {
        "e": nc.declare_dram_parameter("e", [Tc, D], F32, isOutput=False)[:],
        "bo": nc.declare_dram_parameter("bo", [Tc, D], F32, isOutput=False)[:],
        "w": nc.declare_dram_parameter("w", [D, D], F32, isOutput=False)[:],
    }
    if with_h:
        io["h"] = nc.declare_dram_parameter("h", [Tc, D], F32, isOutput=False)[:]
        io["a_raw"] = nc.declare_dram_parameter("a_raw", [1, D], F32, isOutput=False)[:]
    io["out"] = nc.declare_dram_parameter("out", [Tc, D], F32, isOutput=True)[:]
    with tile.TileContext(nc) as tc:
        build_kernel_body(tc, io, Tc, D, with_h)
    legalize_waits(nc)
    return nc


_NC_CACHE: dict = {}


def _get_nc(Tc: int, D: int, with_h: bool):
    key = (Tc, D, with_h)
    if key not in _NC_CACHE:
        _NC_CACHE[key] = build_nc(Tc, D, with_h)
    return _NC_CACHE[key]


def kernel(h, e, block_out, A_raw, W, _trace=False, _trace_kwargs=None):
    Bb, Tt, D = e.shape
    rows = Bb * Tt
    Tc = rows // N_CORES
    e2 = e.reshape(rows, D)
    bo2 = block_out.reshape(rows, D)
    h2 = h.reshape(rows, D)
    with_h = bool(np.any(A_raw))

    nc = _get_nc(Tc, D, with_h)
    in_maps = []
    for c in range(N_CORES):
        sl = slice(c * Tc, (c + 1) * Tc)
        m = {
            "e": np.ascontiguousarray(e2[sl]),
            "bo": np.ascontiguousarray(bo2[sl]),
            "w": np.ascontiguousarray(W),
        }
        if with_h:
            m["h"] = np.ascontiguousarray(h2[sl])
            m["a_raw"] = np.ascontiguousarray(A_raw.reshape(1, D))
        in_maps.append(m)

    res = run_bass_kernel_spmd(
        nc, in_maps, list(range(N_CORES)), trace=_trace,
        **(_trace_kwargs or {}),
    )
    out = np.concatenate([res.results[c]["out"] for c in range(N_CORES)], axis=0)
    if _trace:
        return out.reshape(Bb, Tt, D), res
    return out.reshape(Bb, Tt, D)
